# revision 8
# baseline (speedup 1.0000x reference)
"""Trainium2 Bass kernel for nn_MoEConnectionProcessor (moe_routing).

Computes, for N=19683 cells with K=26 neighbors of state size D=64:
  - per-category masked neighbor means (local / functional / distant)
  - three expert MLPs (SimpleLinear / 2-layer MLP / 3-step Euler CNF)
  - a softmax gating network over [cur, neighbor_activity]
  - count-gated weighted combination

Sharding: pure data parallel over cells across 8 NeuronCores; the tiny
weights are replicated. Inputs are padded from 19683 to 8*2560=20480 rows.
"""

import numpy as np

import concourse.bass as bass
import concourse.tile as tile
import concourse.mybir as mybir
from concourse.bass_utils import run_bass_kernel_spmd

F32 = mybir.dt.float32
I32 = mybir.dt.int32
AX = mybir.AxisListType
OP = mybir.AluOpType
ACTF = mybir.ActivationFunctionType

N_FULL = 19683
K = 26
D = 64
GH = 32
N_CORES = 8
C_CORE = 2560          # cells per core (padded)
TILE = 128             # cells per tile (partition dim)
BLK = 512              # cells per GEMM block
N_BLK = C_CORE // BLK  # 5
T_PER_B = BLK // TILE  # 4
DT_STEP = 1.0 / 3.0


def _split_sync_waits(nc, limit=1):
    """This walrus build rejects instructions carrying more than one sync
    wait ("Too many sync wait commands").  Hoist excess waits onto
    same-engine ENGINE_NOP instructions inserted right before the offender;
    engine sequencers process instructions in order, so semantics are
    preserved."""
    for bb in nc.main_func.blocks:
        insts = list(bb.instructions)
        new = []
        changed = False
        for inst in insts:
            si = getattr(inst, "sync_info", None)
            waits = list(si.on_wait) if si is not None and si.on_wait else []
            if len(waits) > limit:
                changed = True
                cls = type(si)
                head, tail = waits[:-limit], waits[-limit:]
                for w in head:
                    nop = mybir.InstNoOp(
                        name=nc.get_next_instruction_name(), ins=[], outs=[]
                    )
                    nop.engine = inst.engine
                    nop.sync_info = cls(on_update=[], on_wait=[w])
                    nc.register_instruction(nop, overwrite=True)
                    new.append(nop)
                inst.sync_info = cls(on_update=list(si.on_update), on_wait=tail)
            new.append(inst)
        if changed:
            bb.instructions = new


def _bcast_free(ap, count):
    """Append a step-0 free dim of length `count` to an AP (broadcast)."""
    return bass.AP(tensor=ap.tensor, offset=ap.offset, ap=[*ap.ap, [0, count]])


def _rep_free(ap, count, where=1):
    """Insert a step-0 free dim of length `count` after the partition dim."""
    new = list(ap.ap)
    new.insert(where, [0, count])
    return bass.AP(tensor=ap.tensor, offset=ap.offset, ap=new)


def build_kernel():
    nc = bass.Bass("TRN2", target_bir_lowering=False)

    cur_d = nc.dram_tensor("cur", [C_CORE, D], F32, kind="ExternalInput")
    nb_d = nc.dram_tensor("nb", [C_CORE, K, D], F32, kind="ExternalInput")
    cat_d = nc.dram_tensor("cat", [C_CORE, K], I32, kind="ExternalInput")

    w_local_d = nc.dram_tensor("w_local", [2 * D, D], F32, kind="ExternalInput")
    w_f1_d = nc.dram_tensor("w_f1", [2 * D, 2 * D], F32, kind="ExternalInput")
    w_f2_d = nc.dram_tensor("w_f2", [2 * D, D], F32, kind="ExternalInput")
    w_c1_d = nc.dram_tensor("w_c1", [2 * D, 2 * D], F32, kind="ExternalInput")
    w_c2_d = nc.dram_tensor("w_c2", [2 * D, D], F32, kind="ExternalInput")
    w_g1_d = nc.dram_tensor("w_g1", [2 * D, GH], F32, kind="ExternalInput")
    w_g2_d = nc.dram_tensor("w_g2", [GH, 3], F32, kind="ExternalInput")
    b_local_d = nc.dram_tensor("b_local", [D], F32, kind="ExternalInput")
    b_f1_d = nc.dram_tensor("b_f1", [2 * D], F32, kind="ExternalInput")
    b_f2_d = nc.dram_tensor("b_f2", [D], F32, kind="ExternalInput")
    b_c1_d = nc.dram_tensor("b_c1", [2 * D], F32, kind="ExternalInput")
    b_c2_d = nc.dram_tensor("b_c2", [D], F32, kind="ExternalInput")
    b_g1_d = nc.dram_tensor("b_g1", [GH], F32, kind="ExternalInput")
    b_g2_d = nc.dram_tensor("b_g2", [3], F32, kind="ExternalInput")
    ident_d = nc.dram_tensor("ident", [128, 128], F32, kind="ExternalInput")

    comb_d = nc.dram_tensor("comb", [C_CORE, D], F32, kind="ExternalOutput")
    wout_d = nc.dram_tensor("wout", [C_CORE, 3], F32, kind="ExternalOutput")

    with tile.TileContext(nc) as tc:
        with (
            tc.tile_pool(name="singles", bufs=1) as singles,
            tc.tile_pool(name="loads", bufs=3) as loads,
            tc.tile_pool(name="masks", bufs=3) as masksp,
            tc.tile_pool(name="tmps", bufs=2) as tmps,
            tc.tile_pool(name="small", bufs=4) as small,
            tc.tile_pool(name="gzp", bufs=2 * T_PER_B) as gzp,
            tc.tile_pool(name="means", bufs=3) as meansp,
            tc.tile_pool(name="featblk", bufs=2) as featblk,
            tc.tile_pool(name="gemm_sb", bufs=2) as gemm_sb,
            tc.tile_pool(name="outs", bufs=2) as outsp,
            tc.tile_pool(name="ps_sums", bufs=1, space="PSUM") as ps_sums,
            tc.tile_pool(name="ps_tr", bufs=2, space="PSUM") as ps_tr,
            tc.tile_pool(name="ps_gemm", bufs=2, space="PSUM") as ps_gemm,
        ):
            # ---- constants ----
            ident = singles.tile([128, 128], F32)
            nc.sync.dma_start(out=ident, in_=ident_d[:, :])

            # Split-row weights: top/bot halves in separate partition-0 tiles
            # so two-part accumulating GEMMs keep matching base partitions.
            ws = {}
            for name, dram, dout in [
                ("local", w_local_d, D),
                ("f1", w_f1_d, 2 * D),
                ("c1", w_c1_d, 2 * D),
                ("g1", w_g1_d, GH),
            ]:
                top = singles.tile([D, dout], F32, tag=name + "_top")
                nc.sync.dma_start(out=top, in_=dram[0:D, :])
                bot = singles.tile([D, dout], F32, tag=name + "_bot")
                nc.sync.dma_start(out=bot, in_=dram[D:2 * D, :])
                ws[name] = (top, bot)
            for name, dram, kdim, dout in [
                ("f2", w_f2_d, 2 * D, D),
                ("c2", w_c2_d, 2 * D, D),
                ("g2", w_g2_d, GH, 3),
            ]:
                t = singles.tile([kdim, dout], F32, tag=name)
                nc.sync.dma_start(out=t, in_=dram[:, :])
                ws[name] = t

            bs = {}
            for name, dram, dim in [
                ("local", b_local_d, D),
                ("f1", b_f1_d, 2 * D),
                ("f2", b_f2_d, D),
                ("c1", b_c1_d, 2 * D),
                ("c2", b_c2_d, D),
                ("g1", b_g1_d, GH),
                ("g2", b_g2_d, 3),
            ]:
                t = singles.tile([dim, 1], F32, tag="b_" + name)
                nc.sync.dma_start(out=t, in_=dram.rearrange("(d o) -> d o", o=1))
                bs[name] = t

            for g in range(N_BLK):
                curT = featblk.tile([D, BLK], F32, tag="curT")
                mlT = featblk.tile([D, BLK], F32, tag="mlT")
                mfT = featblk.tile([D, BLK], F32, tag="mfT")
                mdT = featblk.tile([D, BLK], F32, tag="mdT")
                actT = featblk.tile([D, BLK], F32, tag="actT")
                gz_tiles = []

                for t in range(T_PER_B):
                    c0 = g * BLK + t * TILE

                    nb_t = loads.tile([TILE, K, D], F32, tag="nb")
                    nc.sync.dma_start(out=nb_t, in_=nb_d[c0:c0 + TILE, :, :])
                    cat_t = loads.tile([TILE, K], I32, tag="cat")
                    nc.sync.dma_start(out=cat_t, in_=cat_d[c0:c0 + TILE, :])
                    cur_t = loads.tile([TILE, D], F32, tag="cur")
                    nc.sync.dma_start(out=cur_t, in_=cur_d[c0:c0 + TILE, :])

                    # masks + counts + recips (cells-major)
                    m0 = masksp.tile([TILE, K], F32, tag="m0")
                    nc.vector.tensor_scalar(m0, cat_t, 0.0, None, OP.is_equal)
                    m1 = masksp.tile([TILE, K], F32, tag="m1")
                    nc.vector.tensor_scalar(m1, cat_t, 1.0, None, OP.is_equal)

                    cnt0 = small.tile([TILE, 1], F32, tag="cnt0")
                    nc.vector.tensor_reduce(cnt0, m0, AX.X, OP.add)
                    cnt1 = small.tile([TILE, 1], F32, tag="cnt1")
                    nc.vector.tensor_reduce(cnt1, m1, AX.X, OP.add)
                    cnt2 = small.tile([TILE, 1], F32, tag="cnt2")
                    # cnt2 = 26 - cnt0 - cnt1 = (cnt0*-1 - cnt1) + 26
                    nc.vector.scalar_tensor_tensor(
                        cnt2, cnt0, -1.0, cnt1, OP.mult, OP.subtract
                    )
                    nc.vector.tensor_scalar(cnt2, cnt2, 26.0, None, OP.add)

                    gz = gzp.tile([TILE, 3], F32, tag="gz")
                    nc.vector.tensor_scalar(gz[:, 0:1], cnt0, 0.0, None, OP.is_gt)
                    nc.vector.tensor_scalar(gz[:, 1:2], cnt1, 0.0, None, OP.is_gt)
                    nc.vector.tensor_scalar(gz[:, 2:3], cnt2, 0.0, None, OP.is_gt)
                    gz_tiles.append(gz)

                    recs = small.tile([TILE, 3], F32, tag="recs")
                    nc.vector.tensor_scalar(recs[:, 0:1], cnt0, 1.0, None, OP.max)
                    nc.vector.tensor_scalar(recs[:, 1:2], cnt1, 1.0, None, OP.max)
                    nc.vector.tensor_scalar(recs[:, 2:3], cnt2, 1.0, None, OP.max)
                    nc.vector.reciprocal(recs, recs)

                    # masked products (the heavy DVE work)
                    tmp0 = tmps.tile([TILE, K, D], F32, tag="tmp0")
                    nc.vector.tensor_tensor(
                        tmp0, nb_t, _bcast_free(m0, D), OP.mult
                    )
                    tmp1 = tmps.tile([TILE, K, D], F32, tag="tmp1")
                    nc.vector.tensor_tensor(
                        tmp1, nb_t, _bcast_free(m1, D), OP.mult
                    )

                    # PE: per-category sums via identity-matmul accumulation
                    def ksum(dst, src):
                        # dst [128, D] psum, src [128, K, D] sbuf
                        nc.tensor.matmul(
                            dst, ident, src[:, 0, :], start=True, stop=False
                        )
                        for k0, k1 in ((1, 9), (9, 17), (17, 25), (25, 26)):
                            out_ap = bass.AP(
                                tensor=dst.tensor,
                                offset=dst.offset,
                                ap=[dst.ap[0], [0, k1 - k0], *dst.ap[1:]],
                            )
                            nc.tensor.matmul(
                                out_ap,
                                ident,
                                src[:, k0:k1, :],
                                start=False,
                                stop=(k1 == K),
                                skip_group_check=True,
                            )

                    s0_ps = ps_sums.tile([TILE, D], F32, tag="s0")
                    ksum(s0_ps, tmp0)
                    s1_ps = ps_sums.tile([TILE, D], F32, tag="s1")
                    ksum(s1_ps, tmp1)
                    tt_ps = ps_sums.tile([TILE, D], F32, tag="tt")
                    ksum(tt_ps, nb_t)

                    # means (cells-major, SBUF)
                    s0_sb = meansp.tile([TILE, D], F32, tag="s0sb")
                    nc.scalar.copy(s0_sb, s0_ps)
                    s1_sb = meansp.tile([TILE, D], F32, tag="s1sb")
                    nc.scalar.copy(s1_sb, s1_ps)
                    sd_sb = meansp.tile([TILE, D], F32, tag="sdsb")
                    # sd = T - s0 - s1  = (s0*-1 - s1) + T
                    nc.vector.scalar_tensor_tensor(
                        sd_sb, s0_sb, -1.0, s1_sb, OP.mult, OP.subtract
                    )
                    nc.vector.tensor_tensor(sd_sb, sd_sb, tt_ps, OP.add)

                    mean_l = meansp.tile([TILE, D], F32, tag="ml")
                    nc.scalar.mul(mean_l, s0_sb, recs[:, 0:1])
                    mean_f = meansp.tile([TILE, D], F32, tag="mf")
                    nc.scalar.mul(mean_f, s1_sb, recs[:, 1:2])
                    mean_d = meansp.tile([TILE, D], F32, tag="md")
                    nc.scalar.mul(mean_d, sd_sb, recs[:, 2:3])
                    act_m = meansp.tile([TILE, D], F32, tag="am")
                    nc.scalar.mul(act_m, tt_ps, 1.0 / K)

                    # transposes to feat-major [D, 128] and copy into block cols
                    for src, dstblk in (
                        (cur_t, curT),
                        (mean_l, mlT),
                        (mean_f, mfT),
                        (mean_d, mdT),
                        (act_m, actT),
                    ):
                        tr_ps = ps_tr.tile([D, TILE], F32, tag="tr")
                        nc.tensor.transpose(tr_ps, src, ident)
                        nc.vector.tensor_copy(
                            dstblk[:, t * TILE:(t + 1) * TILE], tr_ps
                        )

                # ---- GEMM chain, feat-major over the 512-cell block ----
                def gemm2(w, rhs_top, rhs_bot, dout):
                    top, bot = w
                    ps = ps_gemm.tile([dout, BLK], F32, tag="g")
                    nc.tensor.matmul(ps, top, rhs_top, start=True, stop=False)
                    nc.tensor.matmul(ps, bot, rhs_bot, start=False, stop=True)
                    return ps

                def gemm1(w, rhs, dout, kdim):
                    ps = ps_gemm.tile([dout, BLK], F32, tag="g")
                    nc.tensor.matmul(ps, w, rhs, start=True, stop=True)
                    return ps

                # local expert
                zl = gemm2(ws["local"], curT, mlT, D)
                out_l = outsp.tile([D, BLK], F32, tag="outl")
                nc.scalar.activation(out_l, zl, ACTF.Tanh, bias=bs["local"])

                # functional expert
                zf1 = gemm2(ws["f1"], curT, mfT, 2 * D)
                h_f = gemm_sb.tile([2 * D, BLK], F32, tag="hf")
                nc.scalar.activation(h_f, zf1, ACTF.Tanh, bias=bs["f1"])
                zf2 = gemm1(ws["f2"], h_f, D, 2 * D)
                out_f = outsp.tile([D, BLK], F32, tag="outf")
                nc.scalar.activation(out_f, zf2, ACTF.Tanh, bias=bs["f2"])

                # distant expert: 3 Euler steps
                x_cur = curT
                for step in range(3):
                    zc1 = gemm2(ws["c1"], x_cur, mdT, 2 * D)
                    v_sb = gemm_sb.tile([2 * D, BLK], F32, tag="vsb")
                    nc.scalar.activation(v_sb, zc1, ACTF.Tanh, bias=bs["c1"])
                    zc2 = gemm1(ws["c2"], v_sb, D, 2 * D)
                    u_sb = gemm_sb.tile([D, BLK], F32, tag="usb")
                    nc.scalar.activation(u_sb, zc2, ACTF.Tanh, bias=bs["c2"])
                    x_next = outsp.tile([D, BLK], F32, tag="x%d" % step)
                    nc.vector.scalar_tensor_tensor(
                        x_next, u_sb, DT_STEP, x_cur, OP.mult, OP.add
                    )
                    x_cur = x_next
                out_d = x_cur

                # gating
                zg1 = gemm2(ws["g1"], curT, actT, GH)
                h_g = gemm_sb.tile([GH, BLK], F32, tag="hg")
                nc.scalar.activation(h_g, zg1, ACTF.Tanh, bias=bs["g1"])
                zg2 = gemm1(ws["g2"], h_g, 3, GH)
                g_sb = gemm_sb.tile([3, BLK], F32, tag="gsb")
                nc.scalar.activation(
                    g_sb, zg2, ACTF.Identity, bias=bs["g2"]
                )

                # ---- epilogue per tile: softmax + combine (cells-major) ----
                for t in range(T_PER_B):
                    c0 = g * BLK + t * TILE
                    sl = slice(t * TILE, (t + 1) * TILE)

                    gT_ps = ps_tr.tile([TILE, 3], F32, tag="tr")
                    nc.tensor.transpose(gT_ps, g_sb[:, sl], ident[0:3, 0:3])
                    e_sb = small.tile([TILE, 3], F32, tag="esb")
                    nc.scalar.activation(e_sb, gT_ps, ACTF.Exp)
                    ssum = small.tile([TILE, 1], F32, tag="ssum")
                    nc.vector.tensor_reduce(ssum, e_sb, AX.X, OP.add)
                    nc.vector.reciprocal(ssum, ssum)
                    w_sb = small.tile([TILE, 3], F32, tag="wsb")
                    nc.vector.tensor_scalar(w_sb, e_sb, ssum, None, OP.mult)
                    nc.sync.dma_start(out=wout_d[c0:c0 + TILE, :], in_=w_sb)

                    wm = small.tile([TILE, 3], F32, tag="wm")
                    nc.vector.tensor_tensor(wm, w_sb, gz_tiles[t], OP.mult)

                    ol_ps = ps_tr.tile([TILE, D], F32, tag="tr")
                    nc.tensor.transpose(
                        ol_ps, out_l[:, sl], ident[0:D, 0:D]
                    )
                    of_ps = ps_tr.tile([TILE, D], F32, tag="tr")
                    nc.tensor.transpose(
                        of_ps, out_f[:, sl], ident[0:D, 0:D]
                    )
                    od_ps = ps_tr.tile([TILE, D], F32, tag="tr")
                    nc.tensor.transpose(
                        od_ps, out_d[:, sl], ident[0:D, 0:D]
                    )

                    comb = small.tile([TILE, D], F32, tag="comb")
                    nc.vector.tensor_scalar(
                        comb, ol_ps, wm[:, 0:1], None, OP.mult
                    )
                    nc.vector.scalar_tensor_tensor(
                        comb, of_ps, wm[:, 1:2], comb, OP.mult, OP.add
                    )
                    nc.vector.scalar_tensor_tensor(
                        comb, od_ps, wm[:, 2:3], comb, OP.mult, OP.add
                    )
                    nc.sync.dma_start(out=comb_d[c0:c0 + TILE, :], in_=comb)

    _split_sync_waits(nc)
    return nc


_NC_CACHE = None


def _get_nc():
    global _NC_CACHE
    if _NC_CACHE is None:
        _NC_CACHE = build_kernel()
    return _NC_CACHE


def kernel(current_state, neighbor_states, neighbor_cat, W_local, b_local,
           W_f1, b_f1, W_f2, b_f2, W_c1, b_c1, W_c2, b_c2,
           W_g1, b_g1, W_g2, b_g2):
    current_state = np.ascontiguousarray(current_state, dtype=np.float32)
    neighbor_states = np.ascontiguousarray(neighbor_states, dtype=np.float32)
    neighbor_cat = np.ascontiguousarray(neighbor_cat, dtype=np.int32)

    n = current_state.shape[0]
    n_pad = C_CORE * N_CORES
    pad = n_pad - n

    cur_p = np.pad(current_state, ((0, pad), (0, 0)))
    nb_p = np.pad(neighbor_states, ((0, pad), (0, 0), (0, 0)))
    cat_p = np.pad(neighbor_cat, ((0, pad), (0, 0)))

    ident = np.eye(128, dtype=np.float32)
    shared = {
        "w_local": np.ascontiguousarray(W_local, np.float32),
        "w_f1": np.ascontiguousarray(W_f1, np.float32),
        "w_f2": np.ascontiguousarray(W_f2, np.float32),
        "w_c1": np.ascontiguousarray(W_c1, np.float32),
        "w_c2": np.ascontiguousarray(W_c2, np.float32),
        "w_g1": np.ascontiguousarray(W_g1, np.float32),
        "w_g2": np.ascontiguousarray(W_g2, np.float32),
        "b_local": np.ascontiguousarray(b_local, np.float32),
        "b_f1": np.ascontiguousarray(b_f1, np.float32),
        "b_f2": np.ascontiguousarray(b_f2, np.float32),
        "b_c1": np.ascontiguousarray(b_c1, np.float32),
        "b_c2": np.ascontiguousarray(b_c2, np.float32),
        "b_g1": np.ascontiguousarray(b_g1, np.float32),
        "b_g2": np.ascontiguousarray(b_g2, np.float32),
        "ident": ident,
    }

    in_maps = []
    for c in range(N_CORES):
        s = slice(c * C_CORE, (c + 1) * C_CORE)
        m = dict(shared)
        m["cur"] = np.ascontiguousarray(cur_p[s])
        m["nb"] = np.ascontiguousarray(nb_p[s])
        m["cat"] = np.ascontiguousarray(cat_p[s])
        in_maps.append(m)

    nc = _get_nc()
    res = run_bass_kernel_spmd(nc, in_maps, core_ids=list(range(N_CORES)))

    comb = np.concatenate([r["comb"] for r in res.results], axis=0)[:n]
    wout = np.concatenate([r["wout"] for r in res.results], axis=0)[:n]
    return comb, wout


# revision 31
# speedup vs baseline: 258.9913x; 258.9913x over previous
"""Trainium2 Bass kernel for nn_MoEConnectionProcessor (moe_routing).

Computes, for N=19683 cells with K=26 neighbors of state size D=64:
  - per-category masked neighbor means (local / functional / distant)
  - three expert MLPs (SimpleLinear / 2-layer MLP / 3-step Euler CNF)
  - a softmax gating network over [cur, neighbor_activity]
  - count-gated weighted combination

Sharding: pure data parallel over cells across 8 NeuronCores; the tiny
weights are replicated. Inputs are padded from 19683 to 8*2560=20480 rows.

Device pipeline per 128-cell tile:
  - category masks arrive host-packed as int32 (two identical bf16 halves);
    GpSimd broadcast-copies them to [128, K*D] bf16
  - DVE forms the two masked neighbor products in bf16 (2x mode)
  - PE accumulates per-category sums via identity-matmul PSUM accumulation
  - ACT scales sums by 1/count into staging; PE transposes [cur|mean] pairs
    to feature-major; the expert/gating GEMM chain runs at 512-cell blocks
  - epilogue: PE transposes outputs back, ACT exp (+accumulated sum) for the
    softmax, DVE combines experts weighted by count-gated gate weights
"""

import numpy as np

import concourse.bass as bass
import concourse.tile as tile
import concourse.mybir as mybir
from concourse.bass_utils import run_bass_kernel_spmd

F32 = mybir.dt.float32
BF16 = mybir.dt.bfloat16
I32 = mybir.dt.int32
AX = mybir.AxisListType
OP = mybir.AluOpType
ACTF = mybir.ActivationFunctionType

N_FULL = 19683
K = 26
D = 64
GH = 32
N_CORES = 8
C_CORE = 2560          # cells per core (padded)
TILE = 128             # cells per tile (partition dim)
BLK = 512              # cells per GEMM block
N_BLK = C_CORE // BLK  # 5
T_PER_B = BLK // TILE  # 4
DT_STEP = 1.0 / 3.0

# Precision mode for the neighbor aggregation. False = exact fp32 masked
# sums (DVE TT at 1x + one mul on GpSimd); True = bf16 neighbor states
# (half DMA, DVE TT at 2x via GpSimd-expanded packed masks, ~8e-4 absmax
# error from the host-side bf16 cast of neighbor_states).
USE_BF16 = False
NB_DT = BF16 if USE_BF16 else F32


def _split_sync_waits(nc, limit=1):
    """This walrus build rejects instructions carrying more than one sync
    wait ("Too many sync wait commands").  Hoist excess waits onto
    same-engine NoOp instructions inserted right before the offender;
    engine sequencers process instructions in order, so semantics are
    preserved."""
    for bb in nc.main_func.blocks:
        insts = list(bb.instructions)
        new = []
        changed = False
        for inst in insts:
            si = getattr(inst, "sync_info", None)
            waits = list(si.on_wait) if si is not None and si.on_wait else []
            if len(waits) > limit:
                changed = True
                cls = type(si)
                head, tail = waits[:-limit], waits[-limit:]
                for w in head:
                    nop = mybir.InstNoOp(
                        name=nc.get_next_instruction_name(), ins=[], outs=[]
                    )
                    nop.engine = inst.engine
                    nop.sync_info = cls(on_update=[], on_wait=[w])
                    nc.register_instruction(nop, overwrite=True)
                    new.append(nop)
                inst.sync_info = cls(on_update=list(si.on_update), on_wait=tail)
            new.append(inst)
        if changed:
            bb.instructions = new


def _bcast_free(ap, count):
    """Append a step-0 free dim of length `count` to an AP (broadcast)."""
    return bass.AP(tensor=ap.tensor, offset=ap.offset, ap=[*ap.ap, [0, count]])


def build_kernel():
    nc = bass.Bass("TRN2", target_bir_lowering=False)

    cur_d = nc.dram_tensor("cur", [C_CORE, D], F32, kind="ExternalInput")
    nb_d = nc.dram_tensor("nb", [C_CORE, K, D], NB_DT, kind="ExternalInput")
    # meta: [m0p(26) | m1p(26) | recs(3,f32 bits) | gz(3,f32 bits)] per cell
    meta_d = nc.dram_tensor("meta", [C_CORE, 2 * K + 6], I32,
                            kind="ExternalInput")

    # weights, host-preprocessed:
    #   wl:  [W_local_bot; W_local_top]   (X_l = [mlT; curT])
    #   wf1: [W_f1_top; W_f1_bot]         (X_f = [curT; mfT])
    #   wc1: [W_c1_top; W_c1_bot]         (X_c = [xT; mdT], x0 = cur)
    #   wg1: [W_g1_bot; W_g1_top]         (X_g = [actT; curT])
    # all constants in one blob: [ident | wl | wf1 | wf2 | wc1 | wc2 | wg1
    #  | wg2 | 7 bias columns]
    BLOB_COLS = 128 + D + 2 * D + D + 2 * D + D + GH + 3 + 7
    blob_d = nc.dram_tensor("blob", [128, BLOB_COLS], F32,
                            kind="ExternalInput")

    comb_d = nc.dram_tensor("comb", [C_CORE, D], F32, kind="ExternalOutput")
    wout_d = nc.dram_tensor("wout", [C_CORE, 3], F32, kind="ExternalOutput")

    with tile.TileContext(nc) as tc:
        with (
            tc.tile_pool(name="singles", bufs=1) as singles,
            tc.tile_pool(name="loads", bufs=3) as loads,
            tc.tile_pool(name="mexp", bufs=3) as mexp,
            tc.tile_pool(name="tmps", bufs=3) as tmps,
            tc.tile_pool(name="small", bufs=4) as small,
            tc.tile_pool(name="gzrec", bufs=3) as gzrec,
            tc.tile_pool(name="agg", bufs=4) as aggp,
            tc.tile_pool(name="featblk", bufs=3) as featblk,
            tc.tile_pool(name="gemm_sb", bufs=2) as gemm_sb,
            tc.tile_pool(name="outs", bufs=2) as outsp,
            tc.tile_pool(name="ps_sums", bufs=2, space="PSUM") as ps_sums,
            tc.tile_pool(name="ps_tr", bufs=3, space="PSUM") as ps_tr,
            tc.tile_pool(name="ps_gemm", bufs=2, space="PSUM") as ps_gemm,
        ):
            # ---- constants: one DMA ----
            blob = singles.tile([128, BLOB_COLS], F32, tag="blob")
            nc.sync.dma_start(out=blob, in_=blob_d[:, :])
            ident = blob[:, 0:128]
            if USE_BF16:
                ident_s = singles.tile([128, 128], BF16, tag="ident_bf")
                nc.vector.tensor_copy(ident_s, ident)
            else:
                ident_s = ident

            ws, bs = {}, {}
            off = 128
            for name, kdim, dout in [
                ("l", 2 * D, D), ("f1", 2 * D, 2 * D), ("f2", 2 * D, D),
                ("c1", 2 * D, 2 * D), ("c2", 2 * D, D), ("g1", 2 * D, GH),
                ("g2", GH, 3),
            ]:
                ws[name] = blob[0:kdim, off:off + dout]
                off += dout
            for name, dim in zip(
                ["local", "f1", "f2", "c1", "c2", "g1", "g2"],
                [D, 2 * D, D, 2 * D, D, GH, 3],
            ):
                bs[name] = blob[0:dim, off:off + 1]
                off += 1

            def aggregate(g):
                c0b = g * BLK
                # feature-major GEMM inputs for the block
                x_l = featblk.tile([2 * D, BLK], F32, tag="x_l")
                x_f = featblk.tile([2 * D, BLK], F32, tag="x_f")
                x_c = featblk.tile([2 * D, BLK], F32, tag="x_c")
                x_g = featblk.tile([2 * D, BLK], F32, tag="x_g")
                gz_tiles = []

                # block-batched loads for cur/meta; nb per tile (finer deps)
                meta_blk = gzrec.tile([TILE, T_PER_B, 2 * K + 6], I32,
                                      tag="meta")
                nc.sync.dma_start(
                    out=meta_blk,
                    in_=meta_d[c0b:c0b + BLK, :].rearrange(
                        "(t p) d -> p t d", p=TILE
                    ),
                )
                cur_blk = loads.tile([TILE, T_PER_B, D], F32, tag="cur")
                nc.sync.dma_start(
                    out=cur_blk,
                    in_=cur_d[c0b:c0b + BLK, :].rearrange(
                        "(t p) d -> p t d", p=TILE
                    ),
                )

                for t in range(T_PER_B):
                    c0 = g * BLK + t * TILE
                    sl = slice(t * TILE, (t + 1) * TILE)

                    nb_t = loads.tile([TILE, K, D], NB_DT, tag="nb")
                    nc.sync.dma_start(out=nb_t, in_=nb_d[c0:c0 + TILE, :, :])
                    meta_t = meta_blk[:, t, :]
                    recs = meta_t[:, 2 * K:2 * K + 3].bitcast(F32)
                    gz = meta_t[:, 2 * K + 3:2 * K + 6].bitcast(F32)
                    gz_tiles.append(gz)

                    nb_flat = nb_t.rearrange("p a b -> p (a b)")
                    if USE_BF16:
                        # GpSimd expands both packed masks to [128, 2*K*D]
                        # bf16; DVE runs the masked products in 2x mode.
                        me01 = mexp.tile([TILE, K * D], I32, tag="me01")
                        nc.gpsimd.tensor_copy(
                            me01, _bcast_free(meta_t[:, 0:2 * K], D // 2)
                        )
                        me_bf = me01.bitcast(BF16)
                        tmp0 = tmps.tile([TILE, K * D], BF16, tag="tmp0")
                        nc.vector.tensor_tensor(
                            tmp0, nb_flat, me_bf[:, 0:K * D], OP.mult
                        )
                        tmp1 = tmps.tile([TILE, K * D], BF16, tag="tmp1")
                        nc.vector.tensor_tensor(
                            tmp1, nb_flat, me_bf[:, K * D:2 * K * D], OP.mult
                        )
                    else:
                        # fp32 masks ride a step-0 broadcast AP; one product
                        # on DVE, the other on the otherwise-idle GpSimd.
                        m0_b = _bcast_free(
                            meta_t[:, 0:K].bitcast(F32), D
                        )
                        m1_b = _bcast_free(
                            meta_t[:, K:2 * K].bitcast(F32), D
                        )
                        nb3 = nb_t
                        tmp0 = tmps.tile([TILE, K, D], F32, tag="tmp0")
                        nc.vector.tensor_tensor(tmp0, nb3, m0_b, OP.mult)
                        tmp1 = tmps.tile([TILE, K, D], F32, tag="tmp1")
                        nc.gpsimd.tensor_tensor(tmp1, nb3, m1_b, OP.mult)
                        tmp0 = tmp0.rearrange("p a b -> p (a b)")
                        tmp1 = tmp1.rearrange("p a b -> p (a b)")

                    # PE: per-category sums via identity-matmul PSUM
                    # accumulation.  All three sums share one bank as a
                    # single accumulation group: only the very first matmul
                    # clears the bank (start=True); later first-writes to
                    # untouched columns overwrite-and-set-bit per element.
                    sums_ps = ps_sums.tile([TILE, 3 * D], F32, tag="sums")

                    def ksum(dst, src3, first):
                        nc.tensor.matmul(
                            dst, ident_s, src3[:, 0, :],
                            start=first, stop=False,
                            skip_group_check=True,
                        )
                        for k0, k1 in ((1, 9), (9, 17), (17, 25), (25, 26)):
                            out_ap = bass.AP(
                                tensor=dst.tensor,
                                offset=dst.offset,
                                ap=[dst.ap[0], [0, k1 - k0], *dst.ap[1:]],
                            )
                            nc.tensor.matmul(
                                out_ap,
                                ident_s,
                                src3[:, k0:k1, :],
                                start=False,
                                stop=False,
                                skip_group_check=True,
                            )

                    s0_ps = sums_ps[:, 0:D]
                    s1_ps = sums_ps[:, D:2 * D]
                    tt_ps = sums_ps[:, 2 * D:3 * D]
                    ksum(s0_ps, tmp0.rearrange("p (a b) -> p a b", a=K), True)
                    ksum(s1_ps, tmp1.rearrange("p (a b) -> p a b", a=K), False)
                    ksum(tt_ps, nb_t, False)

                    # staging [ml | cur | mf] and [act | cur | md], cells-major
                    agg1 = aggp.tile([TILE, 3 * D], F32, tag="agg1")
                    agg2 = aggp.tile([TILE, 3 * D], F32, tag="agg2")
                    nc.vector.tensor_copy(agg1[:, D:2 * D], cur_blk[:, t, :])
                    nc.scalar.copy(agg2[:, D:2 * D], cur_blk[:, t, :])
                    s0_sb = aggp.tile([TILE, D], F32, tag="s0sb")
                    nc.scalar.copy(s0_sb, s0_ps)
                    # sd_neg = s0 + s1 - T ; mean_d = sd_neg * (-rec2)
                    sd_sb = aggp.tile([TILE, D], F32, tag="sdsb")
                    nc.vector.scalar_tensor_tensor(
                        sd_sb, s0_sb, 1.0, s1_ps, OP.mult, OP.add
                    )
                    nc.vector.scalar_tensor_tensor(
                        sd_sb, sd_sb, 1.0, tt_ps, OP.mult, OP.subtract
                    )

                    nc.scalar.mul(agg1[:, 0:D], s0_ps, recs[:, 0:1])       # ml
                    nc.scalar.mul(agg1[:, 2 * D:3 * D], s1_ps, recs[:, 1:2])  # mf
                    nc.scalar.mul(agg2[:, 0:D], tt_ps, 1.0 / K)            # act
                    nc.scalar.activation(                                   # md
                        agg2[:, 2 * D:3 * D], sd_sb, ACTF.Copy,
                        scale=recs[:, 2:3],
                    )

                    # paired transposes -> feature-major X blocks
                    for src_ap, dstblk, eng in (
                        (agg1[:, 0:2 * D], x_l, "v"),     # [mlT; curT]
                        (agg1[:, D:3 * D], x_f, "s"),     # [curT; mfT]
                        (agg2[:, 0:2 * D], x_g, "v"),     # [actT; curT]
                        (agg2[:, D:3 * D], x_c, "s"),     # [curT; mdT]
                    ):
                        tr_ps = ps_tr.tile([2 * D, TILE], F32, tag="tr")
                        nc.tensor.transpose(tr_ps, src_ap, ident)
                        if eng == "v":
                            nc.vector.tensor_copy(dstblk[:, sl], tr_ps)
                        else:
                            nc.scalar.copy(dstblk[:, sl], tr_ps)

                return x_l, x_f, x_c, x_g, gz_tiles

            def compute(g, state):
                x_l, x_f, x_c, x_g, gz_tiles = state

                # ---- GEMM chain, feature-major over the 512-cell block ----
                def gemm(w, rhs, dout):
                    ps = ps_gemm.tile([dout, BLK], F32, tag="g")
                    nc.tensor.matmul(ps, w, rhs, start=True, stop=True)
                    return ps

                zl = gemm(ws["l"], x_l, D)
                out_l = outsp.tile([D, BLK], F32, tag="outl")
                nc.scalar.activation(out_l, zl, ACTF.Tanh, bias=bs["local"])

                zf1 = gemm(ws["f1"], x_f, 2 * D)
                h_f = gemm_sb.tile([2 * D, BLK], F32, tag="hf")
                nc.scalar.activation(h_f, zf1, ACTF.Tanh, bias=bs["f1"])
                zf2 = gemm(ws["f2"], h_f, D)
                out_f = outsp.tile([D, BLK], F32, tag="outf")
                nc.scalar.activation(out_f, zf2, ACTF.Tanh, bias=bs["f2"])

                # distant expert: 3 Euler steps, x lives in x_c rows 0:D
                for step in range(3):
                    zc1 = gemm(ws["c1"], x_c, 2 * D)
                    v_sb = gemm_sb.tile([2 * D, BLK], F32, tag="vsb")
                    nc.scalar.activation(v_sb, zc1, ACTF.Tanh, bias=bs["c1"])
                    zc2 = gemm(ws["c2"], v_sb, D)
                    u_sb = gemm_sb.tile([D, BLK], F32, tag="usb")
                    nc.scalar.activation(u_sb, zc2, ACTF.Tanh, bias=bs["c2"])
                    nc.vector.scalar_tensor_tensor(
                        x_c[0:D, :], u_sb, DT_STEP, x_c[0:D, :],
                        OP.mult, OP.add,
                    )
                out_d = x_c[0:D, :]

                zg1 = gemm(ws["g1"], x_g, GH)
                h_g = gemm_sb.tile([GH, BLK], F32, tag="hg")
                nc.scalar.activation(h_g, zg1, ACTF.Tanh, bias=bs["g1"])
                zg2 = gemm(ws["g2"], h_g, 3)
                g_sb = gemm_sb.tile([3, BLK], F32, tag="gsb")
                nc.scalar.activation(g_sb, zg2, ACTF.Identity, bias=bs["g2"])

                # ---- epilogue per tile: softmax + combine (cells-major) ----
                comb_blk = outsp.tile([TILE, T_PER_B, D], F32, tag="comb_blk")
                w_blk = outsp.tile([TILE, T_PER_B, 3], F32, tag="w_blk")
                for t in range(T_PER_B):
                    sl = slice(t * TILE, (t + 1) * TILE)

                    gT_ps = ps_tr.tile([TILE, 3], F32, tag="tr")
                    nc.tensor.transpose(gT_ps, g_sb[:, sl], ident[0:3, 0:3])
                    w_sb = w_blk[:, t, :]
                    ssum = small.tile([TILE, 1], F32, tag="ssum")
                    nc.scalar.activation(w_sb, gT_ps, ACTF.Exp, accum_out=ssum)
                    nc.vector.reciprocal(ssum, ssum)
                    wm = small.tile([TILE, 3], F32, tag="wm")
                    # wm = w_raw * gz * (1/sum); w_out = w_raw * (1/sum)
                    nc.vector.tensor_scalar(w_sb, w_sb, ssum, None, OP.mult)
                    nc.vector.tensor_tensor(wm, w_sb, gz_tiles[t], OP.mult)

                    ol_ps = ps_tr.tile([TILE, D], F32, tag="tr")
                    nc.tensor.transpose(ol_ps, out_l[:, sl], ident[0:D, 0:D])
                    of_ps = ps_tr.tile([TILE, D], F32, tag="tr")
                    nc.tensor.transpose(of_ps, out_f[:, sl], ident[0:D, 0:D])
                    od_ps = ps_tr.tile([TILE, D], F32, tag="tr")
                    nc.tensor.transpose(od_ps, out_d[:, sl], ident[0:D, 0:D])

                    comb = comb_blk[:, t, :]
                    nc.vector.tensor_scalar(
                        comb, ol_ps, wm[:, 0:1], None, OP.mult
                    )
                    nc.vector.scalar_tensor_tensor(
                        comb, of_ps, wm[:, 1:2], comb, OP.mult, OP.add
                    )
                    nc.vector.scalar_tensor_tensor(
                        comb, od_ps, wm[:, 2:3], comb, OP.mult, OP.add
                    )
                c0b = g * BLK
                comb_dst = comb_d[c0b:c0b + BLK, :].rearrange(
                    "(t p) d -> p t d", p=TILE
                )
                nc.sync.dma_start(out=comb_dst, in_=comb_blk)
                w_dst = wout_d[c0b:c0b + BLK, :].rearrange(
                    "(t p) d -> p t d", p=TILE
                )
                nc.sync.dma_start(out=w_dst, in_=w_blk)

            # software pipeline: emit aggregation two blocks ahead so its
            # DVE/PE/DMA work is available to overlap the GEMM chain
            states = [aggregate(0), aggregate(1)]
            for g in range(N_BLK):
                if g + 2 < N_BLK:
                    states.append(aggregate(g + 2))
                compute(g, states[g])

    _split_sync_waits(nc)
    return nc


_NC_CACHE = None


def _get_nc():
    global _NC_CACHE
    if _NC_CACHE is None:
        _NC_CACHE = build_kernel()
    return _NC_CACHE


def build_in_maps(current_state, neighbor_states, neighbor_cat, W_local,
                  b_local, W_f1, b_f1, W_f2, b_f2, W_c1, b_c1, W_c2, b_c2,
                  W_g1, b_g1, W_g2, b_g2):
    import ml_dtypes

    current_state = np.ascontiguousarray(current_state, dtype=np.float32)
    neighbor_states = np.ascontiguousarray(neighbor_states, dtype=np.float32)
    neighbor_cat = np.ascontiguousarray(neighbor_cat, dtype=np.int32)

    n = current_state.shape[0]
    n_pad = C_CORE * N_CORES
    pad = n_pad - n

    cur_p = np.pad(current_state, ((0, pad), (0, 0)))
    nb_p = np.pad(neighbor_states, ((0, pad), (0, 0), (0, 0)))
    cat_p = np.pad(neighbor_cat, ((0, pad), (0, 0)))

    if USE_BF16:
        nb_send = nb_p.astype(ml_dtypes.bfloat16)
        # packed masks: int32 whose two bf16 halves are both 1.0 (or 0.0)
        one = np.int32(0x3F803F80)
    else:
        nb_send = nb_p
        one = np.int32(0x3F800000)  # fp32 1.0 bit pattern
    m0p = np.where(cat_p == 0, one, np.int32(0)).astype(np.int32)
    m1p = np.where(cat_p == 1, one, np.int32(0)).astype(np.int32)

    cnt0 = (cat_p == 0).sum(axis=1).astype(np.float32)
    cnt1 = (cat_p == 1).sum(axis=1).astype(np.float32)
    cnt2 = (cat_p == 2).sum(axis=1).astype(np.float32)
    cnts = np.stack([cnt0, cnt1, cnt2], axis=1)
    recs = 1.0 / np.maximum(cnts, 1.0)
    recs[:, 2] = -recs[:, 2]  # mean_d is scaled from (s0+s1-T) by -rec2
    gz = (cnts > 0).astype(np.float32)
    meta = np.concatenate(
        [m0p, m1p, recs.view(np.int32), gz.view(np.int32)], axis=1
    ).astype(np.int32)

    W_local = np.asarray(W_local, np.float32)
    W_g1 = np.asarray(W_g1, np.float32)
    wl = np.concatenate([W_local[D:], W_local[:D]], axis=0)   # [bot; top]
    wg1 = np.concatenate([W_g1[D:], W_g1[:D]], axis=0)        # [bot; top]

    blob_cols = 128 + D + 2 * D + D + 2 * D + D + GH + 3 + 7
    blob = np.zeros((128, blob_cols), np.float32)
    blob[:, 0:128] = np.eye(128, dtype=np.float32)
    off = 128
    for w in [wl, np.asarray(W_f1, np.float32), np.asarray(W_f2, np.float32),
              np.asarray(W_c1, np.float32), np.asarray(W_c2, np.float32),
              wg1, np.asarray(W_g2, np.float32)]:
        blob[0:w.shape[0], off:off + w.shape[1]] = w
        off += w.shape[1]
    for b in [b_local, b_f1, b_f2, b_c1, b_c2, b_g1, b_g2]:
        b = np.asarray(b, np.float32)
        blob[0:b.shape[0], off] = b
        off += 1

    shared = {"blob": blob}

    in_maps = []
    for c in range(N_CORES):
        s = slice(c * C_CORE, (c + 1) * C_CORE)
        m = dict(shared)
        m["cur"] = np.ascontiguousarray(cur_p[s])
        m["nb"] = np.ascontiguousarray(nb_send[s])
        m["meta"] = np.ascontiguousarray(meta[s])
        in_maps.append(m)
    return in_maps


def kernel(**inputs):
    in_maps = build_in_maps(**inputs)
    n = np.asarray(inputs["current_state"]).shape[0]
    nc = _get_nc()
    res = run_bass_kernel_spmd(nc, in_maps, core_ids=list(range(N_CORES)))
    comb = np.concatenate([r["comb"] for r in res.results], axis=0)[:n]
    wout = np.concatenate([r["wout"] for r in res.results], axis=0)[:n]
    return comb, wout
